# revision 41
# baseline (speedup 1.0000x reference)
"""Causal multi-head self-attention on 8 TRN2 NeuronCores.

Sharding: batch (2) x head-group (4 heads = 256 contiguous features) -> 8 cores.
Each core computes q/k/v projections for its 256 output features from its
batch's full activations, then causal attention for its 4 heads. No
collectives: the host concatenates the 8 [S, 256] shards.

Kernel layout choices (v5, host-transposed bf16):
  - the host ships x and W already transposed (xT [D, S], wT [D, HD]) and in
    bf16: the PE transpose stage, its PSUM staging and the psum->sbuf copies
    disappear entirely; SBUF tiles are filled by straight DMA.
  - every matmul runs bf16 -> 1 moving column per PE cycle (measured
    0.545 ns/col streaming rate on this silicon, LDWEIGHTS fully hidden).
    fp8 was evaluated and rejected: attention output is a
    cancellation-dominated sum (z ~ v_rms/sqrt(n_eff)), so independent fp8
    element noise on u or v stays ~4-5% relative and blows the error budget.
  - qT/kT stored [dk, s] (head dim on partitions) so scores come out
    transposed [k, q]; PV consumes exp(scores) directly as the moving
    operand. Row sums come from a ones-column appended to V (65-wide PV
    stationary); normalization happens after a final small bf16 PE
    transpose, deferred and drip-fed into later steps as PE/DVE filler.
  - each key-tile's scores for BOTH chain heads land in one [128, 1024] psum
    tile (head0 at [q0,512), head1 at [512+q0,1024)), so a single wide EXP
    covers two heads and is 1D-contiguous for off-diagonal tiles.
  - causal mask is multiplicative on u after exp (gpsimd, off the ACT/DVE
    critical path): the diagonal 128-block gets a 0/1 tri mask.
  - softmax skips max-subtraction (scores ~ N(0,1) after the exp scale).
  - the whole schedule is emitted as generators explicitly interleaved in
    program order (engine queues are in-order): attention for query group g
    runs against the projection units of s-group g+1. Two head-chains
    pipeline scores/exp/PV; PV lags exp by one step so the PE never waits
    on ACT.
"""

import sys

import numpy as np

sys.path.insert(0, "/opt/trn_rl_repo")

import ml_dtypes

import concourse.bass as bass
import concourse.tile as tile
from concourse import bacc, mybir
from concourse.bass_utils import run_bass_kernel_spmd

B, S, D, H, DK = 2, 2048, 1024, 16, 64
NCORES = 8
HD = 256  # output features per core (4 heads x 64)
NHC = 4  # heads per core
NST = S // 128  # 16 s-tiles
NCC = D // 128  # 8 contraction chunks
NG = S // 512  # 4 query groups of 512

f32 = mybir.dt.float32
f32r = mybir.dt.float32r
bf16 = mybir.dt.bfloat16
AF = mybir.ActivationFunctionType
PSUM = bass.MemorySpace.PSUM


def _body(nc, tc, xt, wqt, wkt, wvt, out, cconst):
    with (
        tc.tile_pool(name="persist", bufs=1) as persist,
        tc.tile_pool(name="u", bufs=6) as u_pool,
        tc.tile_pool(name="zc", bufs=6) as zc_pool,
        tc.tile_pool(name="small", bufs=4) as small,
        tc.tile_pool(name="psum_sp", bufs=2, space=PSUM) as psum_sp,
        tc.tile_pool(name="psum_f", bufs=2, space=PSUM) as psum_f,
        tc.tile_pool(name="psum_acc", bufs=2, space=PSUM) as psum_acc,
    ):
        # one packed const DMA: [identity(128) | tri(128) | ones(128) |
        # bq(2) | bk(2) | bv-row(256, on partition 0)] along the free dim
        cpack = persist.tile([128, 644], bf16)
        nc.sync.dma_start(out=cpack[:], in_=cconst)
        identb = cpack[:, 0:128]
        # tri[k, q] = 1.0 if q >= k else 0.0 (multiplicative causal mask)
        tri = cpack[:, 128:256]
        ones_bf = cpack[:, 256:384]
        bv_row = cpack[0:1, 388:644]  # bv as a row on partition 0
        # q/k biases as f32 (tensor_scalar requires a float32 scalar operand)
        bqk = persist.tile([128, 4], f32)
        nc.scalar.copy(bqk[:], cpack[:, 384:388])

        # preload the Exp activation table while DMAs run
        dummy = small.tile([1, 2], bf16, tag="d", name="dummy")
        nc.scalar.activation(dummy[:], ones_bf[0:1, 0:2], AF.Exp, scale=1.0)

        # ---- weights and xT arrive pre-transposed from the host ----
        wqT = persist.tile([128, NCC, HD], bf16)
        wkT = persist.tile([128, NCC, HD], bf16)
        wvT = persist.tile([128, NCC, HD], bf16)
        xT = persist.tile([128, NCC, S], bf16)  # 32KB/partition

        xt_r = xt.rearrange("(c p) s -> p c s", p=128)

        def dma_xt(sg):
            nc.sync.dma_start(
                out=xT[:, :, bass.ts(sg, 512)], in_=xt_r[:, :, bass.ts(sg, 512)]
            )

        # only the group-0 critical path (wq + xT0) is DMA'd up front so it
        # gets the full HBM bandwidth, chunked per contraction block so the
        # first projection matmul starts after ~200KB; everything else is
        # deferred into the instruction stream
        wq_r = wqt.rearrange("(c p) f -> p c f", p=128)
        for cc in range(NCC):
            nc.sync.dma_start(out=wqT[:, cc, :], in_=wq_r[:, cc, :])
            nc.sync.dma_start(
                out=xT[:, cc, 0:512], in_=xt_r[:, cc, 0:512]
            )

        qT = persist.tile([128, 2, S], bf16)
        kT = persist.tile([128, 2, S], bf16)
        v_aug = persist.tile([128, NST, NHC, 65], bf16)
        z_full = persist.tile([128, NST, HD], bf16)
        nc.scalar.copy(
            v_aug[:, :, :, 64],
            ones_bf[:, 0:64].rearrange("p (a b) -> p a b", a=NST),
        )
        # bv broadcast to 128 rows once (via ones-column rank-1 matmul), so
        # the per-tile v bias rides the psum->sbuf copy as a tensor_tensor add
        pbv = psum_f.tile([128, HD], f32, tag="pf", name="pbv")
        nc.tensor.matmul(
            pbv[:], lhsT=ones_bf[0:1, :], rhs=bv_row, start=True, stop=True
        )
        bv_bc = persist.tile([128, HD], f32)
        nc.vector.tensor_copy(bv_bc[:], pbv[:])

        def gen_fused(sg):
            if sg == 0:
                wk_r = wkt.rearrange("(c p) f -> p c f", p=128)
                for cc in range(NCC):
                    nc.sync.dma_start(out=wkT[:, cc, :], in_=wk_r[:, cc, :])
                nc.sync.dma_start(
                    out=wvT[:], in_=wvt.rearrange("(c p) f -> p c f", p=128)
                )
            if sg + 1 < NG:
                dma_xt(sg + 1)
            # q/k projections for this 512-wide s-chunk, one hdc bank at a
            # time so a psum_f slot frees every ~2us for the deferred
            # z-normalize units
            for wT, bc, dstT in ((wqT, 0, qT), (wkT, 2, kT)):
                for hdc in range(2):
                    pa = psum_f.tile([128, 512], f32, tag="pf", name="pa")
                    for cc in range(NCC):
                        nc.tensor.matmul(
                            pa[:],
                            lhsT=wT[:, cc, bass.ts(hdc, 128)],
                            rhs=xT[:, cc, bass.ts(sg, 512)],
                            start=(cc == 0),
                            stop=(cc == NCC - 1),
                        )
                        if cc == 3:
                            yield
                    nc.vector.tensor_scalar_add(
                        dstT[:, hdc, bass.ts(sg, 512)],
                        pa[:],
                        bqk[:, bc + hdc : bc + hdc + 1],
                    )
                    yield
            # v projection for the 4 s-tiles (pairs, alternating banks)
            for spair in range(2):
                pvs = [
                    psum_f.tile([128, HD], f32, tag="pf", name=f"pv{stl}")
                    for stl in range(2)
                ]
                for cc in range(NCC):
                    for stl in range(2):
                        nc.tensor.matmul(
                            pvs[stl][:],
                            lhsT=xT[:, cc, bass.ts(sg * 4 + spair * 2 + stl, 128)],
                            rhs=wvT[:, cc, :],
                            start=(cc == 0),
                            stop=(cc == NCC - 1),
                        )
                    if cc == 3:
                        yield
                for stl in range(2):
                    st = sg * 4 + spair * 2 + stl
                    nc.vector.tensor_add(
                        v_aug[:, st, :, 0:64],
                        pvs[stl][:].rearrange("p (h d) -> p h d", h=NHC),
                        bv_bc[:].rearrange("p (h d) -> p h d", h=NHC),
                    )
                yield

        def make_qt_unit(g, h, zc, qt):
            def emit():
                zt = psum_f.tile([128, 65], bf16, tag="pf", name="zt")
                nc.tensor.transpose(
                    zt[:], zc[:, bass.ts(qt, 128)], identb[0:65, 0:65]
                )
                r = small.tile([128, 1], f32, tag="r", name="r")
                nc.vector.reciprocal(r[:], zt[:, 64:65])
                nc.vector.tensor_scalar_mul(
                    z_full[:, g * 4 + qt, bass.ts(h, 64)], zt[:, 0:64], r[:]
                )

            return emit

        def make_dma_unit(g, qt):
            def emit():
                st = g * 4 + qt
                nc.sync.dma_start(out=out[bass.ts(st, 128), :], in_=z_full[:, st, :])

            return emit

        def gen_attn(g, pending):
            # Two heads run as interleaved chains. Each step handles one
            # (key-tile, head-pair): two bf16 score matmuls into a flat
            # [128, 1024] psum tile, one wide EXP -> bf16 u, causal mask
            # multiplies on u (gpsimd), and the previous tile's two PV
            # matmuls (so the PE never waits on ACT).
            nkc = 4 * g + 4
            for hp in range(0, NHC, 2):
                heads = (hp, hp + 1)
                zps = {
                    h: psum_acc.tile([65, 512], f32, tag="acc", name=f"zp{h}")
                    for h in heads
                }
                def flush_pv(tile_):
                    u, kc, q0 = tile_
                    for ci, h in enumerate(heads):
                        nc.tensor.matmul(
                            zps[h][:, q0:512],
                            lhsT=v_aug[:, kc, h, :],
                            rhs=u[:, bass.ds(512 * ci + q0, 512 - q0)],
                            start=(kc == 0),
                            stop=(kc == nkc - 1),
                        )

                def emit_exp(st1):
                    # exp + mask for a score tile from the previous step:
                    # its matmuls finished a full step ago, so ACT never
                    # stalls on the PE
                    sp, kc, q0 = st1
                    j = kc - 4 * g
                    u = u_pool.tile([128, 1024], bf16, tag="u", name="u")
                    if q0 == 0:
                        nc.scalar.activation(u[:], sp[:], AF.Exp, scale=0.125)
                    else:
                        nc.scalar.activation(
                            u.rearrange("p (c q) -> p c q", c=2)[:, :, q0:512],
                            sp.rearrange("p (c q) -> p c q", c=2)[:, :, q0:512],
                            AF.Exp,
                            scale=0.125,
                        )
                    if j >= 0:
                        d0 = 128 * j
                        for ci in range(2):
                            nc.gpsimd.tensor_mul(
                                u[:, bass.ds(512 * ci + d0, 128)],
                                u[:, bass.ds(512 * ci + d0, 128)],
                                tri[:],
                            )
                    return (u, kc, q0)

                s1 = None  # scored last step, exp this step
                s2 = None  # exp'd last step, PV this step
                for kc in range(nkc):
                    for _ in range(2):
                        if pending:
                            pending.popleft()()
                    j = kc - 4 * g
                    q0 = max(0, 128 * j)
                    sp = psum_sp.tile([128, 1024], f32, tag="sp", name="sp")
                    for ci, h in enumerate(heads):
                        po = (h % 2) * 64
                        hdc = h // 2
                        nc.tensor.matmul(
                            sp[:, bass.ds(512 * ci + q0, 512 - q0)],
                            lhsT=kT[po : po + 64, hdc, bass.ts(kc, 128)],
                            rhs=qT[po : po + 64, hdc, bass.ds(g * 512 + q0, 512 - q0)],
                            start=True,
                            stop=True,
                        )
                    nxt = emit_exp(s1) if s1 is not None else None
                    if s2 is not None:
                        flush_pv(s2)
                    s2 = nxt
                    s1 = (sp, kc, q0)
                    yield
                nxt = emit_exp(s1)
                if s2 is not None:
                    flush_pv(s2)
                flush_pv(nxt)

                # row-sum copy frees the zp slot now; the per-qtile
                # transpose/normalize units are deferred so they interleave
                # into later steps as PE/DVE filler instead of stalling here
                zcs = {}
                for h in heads:
                    zc = zc_pool.tile([65, 512], bf16, tag="zc", name="zc")
                    nc.vector.tensor_copy(zc[:], zps[h][:])
                    zcs[h] = zc
                if g == NG - 1 and hp == 2:
                    # final drain: qt-major with the output DMA interleaved
                    # so stores overlap the remaining normalize work
                    for qt in range(4):
                        for h in heads:
                            pending.append(make_qt_unit(g, h, zcs[h], qt))
                        pending.append(make_dma_unit(g, qt))
                else:
                    for h in heads:
                        for qt in range(4):
                            pending.append(make_qt_unit(g, h, zcs[h], qt))
                yield
            if g != NG - 1:
                for qt in range(4):
                    pending.append(make_dma_unit(g, qt))

        # explicit program-order interleave: attention for group g alternates
        # with the projection units of s-group g+1, so every engine queue
        # mixes both work streams
        from collections import deque
        from itertools import chain as ichain

        pending = deque()
        # run fused(0) through its q/k projections, then let attention(0)
        # start against its remaining v units
        f0 = gen_fused(0)
        for _ in range(8):
            next(f0)
        for sg in range(NG):
            a = gen_attn(sg, pending)
            if sg == 0:
                f = ichain(f0, gen_fused(1))
            elif sg + 1 < NG:
                f = gen_fused(sg + 1)
            else:
                f = iter(())
            while True:
                sa = next(a, StopIteration)
                sf = next(f, StopIteration)
                if sa is StopIteration and sf is StopIteration:
                    break
        while pending:
            pending.popleft()()


def build():
    nc = bacc.Bacc(
        "TRN2", target_bir_lowering=False, debug=False, num_devices=NCORES
    )
    xt = nc.dram_tensor("xt", [D, S], bf16, kind="ExternalInput")
    wqt = nc.dram_tensor("wqt", [D, HD], bf16, kind="ExternalInput")
    wkt = nc.dram_tensor("wkt", [D, HD], bf16, kind="ExternalInput")
    wvt = nc.dram_tensor("wvt", [D, HD], bf16, kind="ExternalInput")
    cconst = nc.dram_tensor("cconst", [128, 644], bf16, kind="ExternalInput")
    out = nc.dram_tensor("out", [S, HD], bf16, kind="ExternalOutput")
    with tile.TileContext(nc) as tc:
        _body(nc, tc, xt.ap(), wqt.ap(), wkt.ap(), wvt.ap(), out.ap(), cconst.ap())
    nc.compile()
    return nc


_NC_CACHE = None


def _get_nc():
    global _NC_CACHE
    if _NC_CACHE is None:
        _NC_CACHE = build()
    return _NC_CACHE


def make_in_maps(q_input, W_q, b_q, W_k, b_k, W_v, b_v):
    bf = ml_dtypes.bfloat16
    ii = np.arange(128)
    # host-side layout prep: transpose x and W so the kernel's contraction
    # dim lands on SBUF partitions with no on-device transposes
    xtb = [
        np.ascontiguousarray(np.asarray(q_input[b], dtype=np.float32).T).astype(bf)
        for b in range(B)
    ]
    wqt = np.asarray(W_q, dtype=np.float32).T.astype(bf)
    wkt = np.asarray(W_k, dtype=np.float32).T.astype(bf)
    wvt = np.asarray(W_v, dtype=np.float32).T.astype(bf)
    in_maps = []
    for c in range(NCORES):
        b = c // 4
        hs = slice((c % 4) * HD, (c % 4 + 1) * HD)
        cconst = np.zeros((128, 644), np.float32)
        cconst[:, 0:128] = np.eye(128, dtype=np.float32)
        cconst[:, 128:256] = ii[None, :] >= ii[:, None]
        cconst[:, 256:384] = 1.0
        bqs = np.asarray(b_q[hs], dtype=np.float32)
        bks = np.asarray(b_k[hs], dtype=np.float32)
        cconst[:, 384] = bqs[0:128]
        cconst[:, 385] = bqs[128:256]
        cconst[:, 386] = bks[0:128]
        cconst[:, 387] = bks[128:256]
        cconst[0, 388:644] = np.asarray(b_v[hs], dtype=np.float32)
        in_maps.append(
            {
                "xt": xtb[b],
                "wqt": np.ascontiguousarray(wqt[:, hs]),
                "wkt": np.ascontiguousarray(wkt[:, hs]),
                "wvt": np.ascontiguousarray(wvt[:, hs]),
                "cconst": cconst.astype(bf),
            }
        )
    return in_maps


def assemble(results):
    full = np.empty((B, S, D), dtype=np.float32)
    for c in range(NCORES):
        b = c // 4
        hs = slice((c % 4) * HD, (c % 4 + 1) * HD)
        full[b, :, hs] = np.asarray(results[c]["out"], dtype=np.float32)
    return full


def _ensure_ntff_hook():
    """Register the axon NTFF profiling hook if the image's antenv lacks it."""
    try:
        from antenv import axon_hooks  # noqa: F401

        return
    except ImportError:
        pass
    import types

    try:
        from trn_agent_boot.trn_boot import _ntff_profile_via_ctypes

        hook = _ntff_profile_via_ctypes("/opt/axon/libaxon_pjrt.so")
    except Exception:
        hook = None
    mod = types.ModuleType("antenv.axon_hooks")
    mod._hook = hook
    mod.get_axon_ntff_profile_hook = lambda: mod._hook

    def _set(h):
        mod._hook = h

    mod.set_axon_ntff_profile_hook = _set
    sys.modules["antenv.axon_hooks"] = mod
    try:
        import antenv

        antenv.axon_hooks = mod
    except ImportError:
        pass


def run(inputs_dict, trace=False):
    """Run on hardware; returns (full_output, BassKernelResults)."""
    nc = _get_nc()
    if trace:
        _ensure_ntff_hook()
        import concourse.bass_utils as _bu

        _bu.upload_artifacts = lambda d: d  # no bucket access in this env
    in_maps = make_in_maps(**{k: np.asarray(v) for k, v in inputs_dict.items()})
    res = run_bass_kernel_spmd(nc, in_maps, core_ids=list(range(NCORES)), trace=trace)
    return assemble(res.results), res


def kernel(**inputs):
    out, _ = run(inputs, trace=False)
    return out


# revision 43
# speedup vs baseline: 1.0040x; 1.0040x over previous
"""Causal multi-head self-attention on 8 TRN2 NeuronCores.

Sharding: batch (2) x head-group (4 heads = 256 contiguous features) -> 8 cores.
Each core computes q/k/v projections for its 256 output features from its
batch's full activations, then causal attention for its 4 heads. No
collectives: the host concatenates the 8 [S, 256] shards.

Kernel layout choices (v5, host-transposed bf16):
  - the host ships x and W already transposed (xT [D, S], wT [D, HD]) and in
    bf16: the PE transpose stage, its PSUM staging and the psum->sbuf copies
    disappear entirely; SBUF tiles are filled by straight DMA.
  - every matmul runs bf16 -> 1 moving column per PE cycle (measured
    0.545 ns/col streaming rate on this silicon, LDWEIGHTS fully hidden).
    fp8 was evaluated and rejected: attention output is a
    cancellation-dominated sum (z ~ v_rms/sqrt(n_eff)), so independent fp8
    element noise on u or v stays ~4-5% relative and blows the error budget.
  - qT/kT stored [dk, s] (head dim on partitions) so scores come out
    transposed [k, q]; PV consumes exp(scores) directly as the moving
    operand. Row sums come from a ones-column appended to V (65-wide PV
    stationary); normalization happens after a final small bf16 PE
    transpose, deferred and drip-fed into later steps as PE/DVE filler.
  - each key-tile's scores for BOTH chain heads land in one [128, 1024] psum
    tile (head0 at [q0,512), head1 at [512+q0,1024)), so a single wide EXP
    covers two heads and is 1D-contiguous for off-diagonal tiles.
  - causal mask is multiplicative on u after exp (gpsimd, off the ACT/DVE
    critical path): the diagonal 128-block gets a 0/1 tri mask.
  - softmax skips max-subtraction (scores ~ N(0,1) after the exp scale).
  - the whole schedule is emitted as generators explicitly interleaved in
    program order (engine queues are in-order): attention for query group g
    runs against the projection units of s-group g+1. Two head-chains
    pipeline scores/exp/PV; PV lags exp by one step so the PE never waits
    on ACT.
"""

import sys

import numpy as np

sys.path.insert(0, "/opt/trn_rl_repo")

import ml_dtypes

import concourse.bass as bass
import concourse.tile as tile
from concourse import bacc, mybir
from concourse.bass_utils import run_bass_kernel_spmd

B, S, D, H, DK = 2, 2048, 1024, 16, 64
NCORES = 8
HD = 256  # output features per core (4 heads x 64)
NHC = 4  # heads per core
NST = S // 128  # 16 s-tiles
NCC = D // 128  # 8 contraction chunks
NG = S // 512  # 4 query groups of 512

f32 = mybir.dt.float32
f32r = mybir.dt.float32r
bf16 = mybir.dt.bfloat16
AF = mybir.ActivationFunctionType
PSUM = bass.MemorySpace.PSUM


def _body(nc, tc, xt, wqt, wkt, wvt, out, cconst):
    with (
        tc.tile_pool(name="persist", bufs=1) as persist,
        tc.tile_pool(name="u", bufs=6) as u_pool,
        tc.tile_pool(name="zc", bufs=6) as zc_pool,
        tc.tile_pool(name="small", bufs=4) as small,
        tc.tile_pool(name="psum_sp", bufs=2, space=PSUM) as psum_sp,
        tc.tile_pool(name="psum_f", bufs=2, space=PSUM) as psum_f,
        tc.tile_pool(name="psum_acc", bufs=2, space=PSUM) as psum_acc,
    ):
        # one packed const DMA: [identity(128) | tri(128) | ones(128) |
        # bq(2) | bk(2) | bv-row(256, on partition 0)] along the free dim
        cpack = persist.tile([128, 644], bf16)
        nc.sync.dma_start(out=cpack[:], in_=cconst)
        identb = cpack[:, 0:128]
        # tri[k, q] = 1.0 if q >= k else 0.0 (multiplicative causal mask)
        tri = cpack[:, 128:256]
        ones_bf = cpack[:, 256:384]
        bv_row = cpack[0:1, 388:644]  # bv as a row on partition 0
        # q/k biases as f32 (tensor_scalar requires a float32 scalar operand)
        bqk = persist.tile([128, 4], f32)
        nc.scalar.copy(bqk[:], cpack[:, 384:388])

        # preload the Exp activation table while DMAs run
        dummy = small.tile([1, 2], bf16, tag="d", name="dummy")
        nc.scalar.activation(dummy[:], ones_bf[0:1, 0:2], AF.Exp, scale=1.0)

        # ---- weights and xT arrive pre-transposed from the host ----
        wqT = persist.tile([128, NCC, HD], bf16)
        wkT = persist.tile([128, NCC, HD], bf16)
        wvT = persist.tile([128, NCC, HD], bf16)
        xT = persist.tile([128, NCC, S], bf16)  # 32KB/partition

        xt_r = xt.rearrange("(c p) s -> p c s", p=128)

        def dma_xt(sg):
            nc.sync.dma_start(
                out=xT[:, :, bass.ts(sg, 512)], in_=xt_r[:, :, bass.ts(sg, 512)]
            )

        # only the group-0 critical path (wq + xT0) is DMA'd up front so it
        # gets the full HBM bandwidth, chunked per contraction block so the
        # first projection matmul starts after ~200KB; everything else is
        # deferred into the instruction stream
        wq_r = wqt.rearrange("(c p) f -> p c f", p=128)
        for cc in range(NCC):
            nc.sync.dma_start(out=wqT[:, cc, :], in_=wq_r[:, cc, :])
            nc.sync.dma_start(
                out=xT[:, cc, 0:512], in_=xt_r[:, cc, 0:512]
            )

        qT = persist.tile([128, 2, S], bf16)
        kT = persist.tile([128, 2, S], bf16)
        v_aug = persist.tile([128, NST, NHC, 65], bf16)
        z_full = persist.tile([128, NST, HD], bf16)
        nc.scalar.copy(
            v_aug[:, :, :, 64],
            ones_bf[:, 0:64].rearrange("p (a b) -> p a b", a=NST),
        )
        # bv broadcast to 128 rows once (via ones-column rank-1 matmul), so
        # the per-tile v bias rides the psum->sbuf copy as a tensor_tensor add
        pbv = psum_f.tile([128, HD], f32, tag="pf", name="pbv")
        nc.tensor.matmul(
            pbv[:], lhsT=ones_bf[0:1, :], rhs=bv_row, start=True, stop=True
        )
        bv_bc = persist.tile([128, HD], f32)
        nc.vector.tensor_copy(bv_bc[:], pbv[:])

        def gen_fused(sg):
            if sg == 0:
                nc.sync.dma_start(
                    out=wkT[:], in_=wkt.rearrange("(c p) f -> p c f", p=128)
                )
                nc.sync.dma_start(
                    out=wvT[:], in_=wvt.rearrange("(c p) f -> p c f", p=128)
                )
            if sg + 1 < NG:
                dma_xt(sg + 1)
            # q/k projections for this 512-wide s-chunk, one hdc bank at a
            # time so a psum_f slot frees every ~2us for the deferred
            # z-normalize units
            for wT, bc, dstT in ((wqT, 0, qT), (wkT, 2, kT)):
                for hdc in range(2):
                    pa = psum_f.tile([128, 512], f32, tag="pf", name="pa")
                    for cc in range(NCC):
                        nc.tensor.matmul(
                            pa[:],
                            lhsT=wT[:, cc, bass.ts(hdc, 128)],
                            rhs=xT[:, cc, bass.ts(sg, 512)],
                            start=(cc == 0),
                            stop=(cc == NCC - 1),
                        )
                        if cc == 3:
                            yield
                    nc.vector.tensor_scalar_add(
                        dstT[:, hdc, bass.ts(sg, 512)],
                        pa[:],
                        bqk[:, bc + hdc : bc + hdc + 1],
                    )
                    yield
            # v projection for the 4 s-tiles (pairs, alternating banks)
            for spair in range(2):
                pvs = [
                    psum_f.tile([128, HD], f32, tag="pf", name=f"pv{stl}")
                    for stl in range(2)
                ]
                for cc in range(NCC):
                    for stl in range(2):
                        nc.tensor.matmul(
                            pvs[stl][:],
                            lhsT=xT[:, cc, bass.ts(sg * 4 + spair * 2 + stl, 128)],
                            rhs=wvT[:, cc, :],
                            start=(cc == 0),
                            stop=(cc == NCC - 1),
                        )
                    if cc == 3:
                        yield
                for stl in range(2):
                    st = sg * 4 + spair * 2 + stl
                    nc.vector.tensor_add(
                        v_aug[:, st, :, 0:64],
                        pvs[stl][:].rearrange("p (h d) -> p h d", h=NHC),
                        bv_bc[:].rearrange("p (h d) -> p h d", h=NHC),
                    )
                yield

        def make_qt_unit(g, h, zc, qt):
            def emit():
                zt = psum_f.tile([128, 65], bf16, tag="pf", name="zt")
                nc.tensor.transpose(
                    zt[:], zc[:, bass.ts(qt, 128)], identb[0:65, 0:65]
                )
                r = small.tile([128, 1], f32, tag="r", name="r")
                nc.vector.reciprocal(r[:], zt[:, 64:65])
                nc.vector.tensor_scalar_mul(
                    z_full[:, g * 4 + qt, bass.ts(h, 64)], zt[:, 0:64], r[:]
                )

            return emit

        def make_dma_unit(g, qt):
            def emit():
                st = g * 4 + qt
                nc.sync.dma_start(out=out[bass.ts(st, 128), :], in_=z_full[:, st, :])

            return emit

        def gen_attn(g, pending):
            # Two heads run as interleaved chains. Each step handles one
            # (key-tile, head-pair): two bf16 score matmuls into a flat
            # [128, 1024] psum tile, one wide EXP -> bf16 u, causal mask
            # multiplies on u (gpsimd), and the previous tile's two PV
            # matmuls (so the PE never waits on ACT).
            nkc = 4 * g + 4
            for hp in range(0, NHC, 2):
                heads = (hp, hp + 1)
                zps = {
                    h: psum_acc.tile([65, 512], f32, tag="acc", name=f"zp{h}")
                    for h in heads
                }
                def flush_pv(tile_):
                    u, kc, q0 = tile_
                    for ci, h in enumerate(heads):
                        nc.tensor.matmul(
                            zps[h][:, q0:512],
                            lhsT=v_aug[:, kc, h, :],
                            rhs=u[:, bass.ds(512 * ci + q0, 512 - q0)],
                            start=(kc == 0),
                            stop=(kc == nkc - 1),
                        )

                def emit_exp(st1):
                    # exp + mask for a score tile from the previous step:
                    # its matmuls finished a full step ago, so ACT never
                    # stalls on the PE
                    sp, kc, q0 = st1
                    j = kc - 4 * g
                    u = u_pool.tile([128, 1024], bf16, tag="u", name="u")
                    if q0 == 0:
                        nc.scalar.activation(u[:], sp[:], AF.Exp, scale=0.125)
                    else:
                        nc.scalar.activation(
                            u.rearrange("p (c q) -> p c q", c=2)[:, :, q0:512],
                            sp.rearrange("p (c q) -> p c q", c=2)[:, :, q0:512],
                            AF.Exp,
                            scale=0.125,
                        )
                    if j >= 0:
                        d0 = 128 * j
                        for ci in range(2):
                            nc.gpsimd.tensor_mul(
                                u[:, bass.ds(512 * ci + d0, 128)],
                                u[:, bass.ds(512 * ci + d0, 128)],
                                tri[:],
                            )
                    return (u, kc, q0)

                s1 = None  # scored last step, exp this step
                s2 = None  # exp'd last step, PV this step
                for kc in range(nkc):
                    for _ in range(2):
                        if pending:
                            pending.popleft()()
                    j = kc - 4 * g
                    q0 = max(0, 128 * j)
                    sp = psum_sp.tile([128, 1024], f32, tag="sp", name="sp")
                    for ci, h in enumerate(heads):
                        po = (h % 2) * 64
                        hdc = h // 2
                        nc.tensor.matmul(
                            sp[:, bass.ds(512 * ci + q0, 512 - q0)],
                            lhsT=kT[po : po + 64, hdc, bass.ts(kc, 128)],
                            rhs=qT[po : po + 64, hdc, bass.ds(g * 512 + q0, 512 - q0)],
                            start=True,
                            stop=True,
                        )
                    nxt = emit_exp(s1) if s1 is not None else None
                    if s2 is not None:
                        flush_pv(s2)
                    s2 = nxt
                    s1 = (sp, kc, q0)
                    yield
                nxt = emit_exp(s1)
                if s2 is not None:
                    flush_pv(s2)
                flush_pv(nxt)

                # row-sum copy frees the zp slot now; the per-qtile
                # transpose/normalize units are deferred so they interleave
                # into later steps as PE/DVE filler instead of stalling here
                zcs = {}
                for h in heads:
                    zc = zc_pool.tile([65, 512], bf16, tag="zc", name="zc")
                    nc.vector.tensor_copy(zc[:], zps[h][:])
                    zcs[h] = zc
                if g == NG - 1 and hp == 2:
                    # final drain: qt-major with the output DMA interleaved
                    # so stores overlap the remaining normalize work
                    for qt in range(4):
                        for h in heads:
                            pending.append(make_qt_unit(g, h, zcs[h], qt))
                        pending.append(make_dma_unit(g, qt))
                else:
                    for h in heads:
                        for qt in range(4):
                            pending.append(make_qt_unit(g, h, zcs[h], qt))
                yield
            if g != NG - 1:
                for qt in range(4):
                    pending.append(make_dma_unit(g, qt))

        # explicit program-order interleave: attention for group g alternates
        # with the projection units of s-group g+1, so every engine queue
        # mixes both work streams
        from collections import deque

        pending = deque()
        for _ in gen_fused(0):
            pass
        for sg in range(NG):
            a = gen_attn(sg, pending)
            f = gen_fused(sg + 1) if sg + 1 < NG else iter(())
            while True:
                sa = next(a, StopIteration)
                sf = next(f, StopIteration)
                if sa is StopIteration and sf is StopIteration:
                    break
        while pending:
            pending.popleft()()


def build():
    nc = bacc.Bacc(
        "TRN2", target_bir_lowering=False, debug=False, num_devices=NCORES
    )
    xt = nc.dram_tensor("xt", [D, S], bf16, kind="ExternalInput")
    wqt = nc.dram_tensor("wqt", [D, HD], bf16, kind="ExternalInput")
    wkt = nc.dram_tensor("wkt", [D, HD], bf16, kind="ExternalInput")
    wvt = nc.dram_tensor("wvt", [D, HD], bf16, kind="ExternalInput")
    cconst = nc.dram_tensor("cconst", [128, 644], bf16, kind="ExternalInput")
    out = nc.dram_tensor("out", [S, HD], bf16, kind="ExternalOutput")
    with tile.TileContext(nc) as tc:
        _body(nc, tc, xt.ap(), wqt.ap(), wkt.ap(), wvt.ap(), out.ap(), cconst.ap())
    nc.compile()
    return nc


_NC_CACHE = None


def _get_nc():
    global _NC_CACHE
    if _NC_CACHE is None:
        _NC_CACHE = build()
    return _NC_CACHE


def make_in_maps(q_input, W_q, b_q, W_k, b_k, W_v, b_v):
    bf = ml_dtypes.bfloat16
    ii = np.arange(128)
    # host-side layout prep: transpose x and W so the kernel's contraction
    # dim lands on SBUF partitions with no on-device transposes
    xtb = [
        np.ascontiguousarray(np.asarray(q_input[b], dtype=np.float32).T).astype(bf)
        for b in range(B)
    ]
    wqt = np.asarray(W_q, dtype=np.float32).T.astype(bf)
    wkt = np.asarray(W_k, dtype=np.float32).T.astype(bf)
    wvt = np.asarray(W_v, dtype=np.float32).T.astype(bf)
    in_maps = []
    for c in range(NCORES):
        b = c // 4
        hs = slice((c % 4) * HD, (c % 4 + 1) * HD)
        cconst = np.zeros((128, 644), np.float32)
        cconst[:, 0:128] = np.eye(128, dtype=np.float32)
        cconst[:, 128:256] = ii[None, :] >= ii[:, None]
        cconst[:, 256:384] = 1.0
        bqs = np.asarray(b_q[hs], dtype=np.float32)
        bks = np.asarray(b_k[hs], dtype=np.float32)
        cconst[:, 384] = bqs[0:128]
        cconst[:, 385] = bqs[128:256]
        cconst[:, 386] = bks[0:128]
        cconst[:, 387] = bks[128:256]
        cconst[0, 388:644] = np.asarray(b_v[hs], dtype=np.float32)
        in_maps.append(
            {
                "xt": xtb[b],
                "wqt": np.ascontiguousarray(wqt[:, hs]),
                "wkt": np.ascontiguousarray(wkt[:, hs]),
                "wvt": np.ascontiguousarray(wvt[:, hs]),
                "cconst": cconst.astype(bf),
            }
        )
    return in_maps


def assemble(results):
    full = np.empty((B, S, D), dtype=np.float32)
    for c in range(NCORES):
        b = c // 4
        hs = slice((c % 4) * HD, (c % 4 + 1) * HD)
        full[b, :, hs] = np.asarray(results[c]["out"], dtype=np.float32)
    return full


def _ensure_ntff_hook():
    """Register the axon NTFF profiling hook if the image's antenv lacks it."""
    try:
        from antenv import axon_hooks  # noqa: F401

        return
    except ImportError:
        pass
    import types

    try:
        from trn_agent_boot.trn_boot import _ntff_profile_via_ctypes

        hook = _ntff_profile_via_ctypes("/opt/axon/libaxon_pjrt.so")
    except Exception:
        hook = None
    mod = types.ModuleType("antenv.axon_hooks")
    mod._hook = hook
    mod.get_axon_ntff_profile_hook = lambda: mod._hook

    def _set(h):
        mod._hook = h

    mod.set_axon_ntff_profile_hook = _set
    sys.modules["antenv.axon_hooks"] = mod
    try:
        import antenv

        antenv.axon_hooks = mod
    except ImportError:
        pass


def run(inputs_dict, trace=False):
    """Run on hardware; returns (full_output, BassKernelResults)."""
    nc = _get_nc()
    if trace:
        _ensure_ntff_hook()
        import concourse.bass_utils as _bu

        _bu.upload_artifacts = lambda d: d  # no bucket access in this env
    in_maps = make_in_maps(**{k: np.asarray(v) for k, v in inputs_dict.items()})
    res = run_bass_kernel_spmd(nc, in_maps, core_ids=list(range(NCORES)), trace=trace)
    return assemble(res.results), res


def kernel(**inputs):
    out, _ = run(inputs, trace=False)
    return out
